# revision 20
# baseline (speedup 1.0000x reference)
"""NT-Xent loss kernel for Trainium2, 8-core SPMD — v6 (symmetric triangle).

Math (matches the reference):
  reps = concat(z_i, z_j)  [2B, C], B=4096, C=128; rhat = reps/|reps|
  S_r = sum_c exp(2 rhat_r . rhat_c);  pos_r = rhat_r . rhat_{(r+B)%2B}
  loss = mean_r( ln(S_r - e^2 + e^{2 pos_r}) - 2 pos_r )

exp(2 sim) is symmetric, so each unordered pair is exp'd ONCE:
  Rotation decomposition of the 64x64 block grid (blocks of 128 rows):
  every core runs the SAME canonical program computing tiles (a, a+d),
  a in 0..7, d in 0..31, on keys ROTATED by 8k blocks (host np.roll).
  Across 8 cores this covers every pair with |d| in 0..31 exactly once.
  The d=32 pairs (also the positive pairs) don't decompose under the
  rotation group; they are fed as separate per-core gathered inputs
  q32/p32 (4 tiles per core), whose diagonals give pos_r for free.

Per tile both sums are extracted: row sums ride ScalarE's accum_out;
column sums are accumulated elementwise into a bf16 strip A (DVE adds)
and partition-folded ONCE at the end on the PE (per-tile lhsT=A-tile
matmul against a ones vector). Host scatter-adds the per-core partials
(static maps), subtracts e^2 for the diagonal, and takes ln/mean in
fp64 over the 8192 rows.

Per core: 260 exp tiles (vs 512 full-matrix), 2.44MB keys DMA (only
blocks 0..38 are referenced canonically), bf16 matmuls (1024-wide
moving operand), normalization folded into the transposed copy via a
partition-broadcast row of 1/|k| (one DVE pass).
"""

import os

import numpy as np

import concourse.bacc as bacc
import concourse.bass as bass
import concourse.mybir as mybir
from concourse.bass_utils import run_bass_kernel_spmd
from concourse.masks import make_identity
from concourse.tile import TileContext

F32 = mybir.dt.float32
F32R = mybir.dt.float32r
BF16 = mybir.dt.bfloat16
AF = mybir.ActivationFunctionType
ALU = mybir.AluOpType
AX = mybir.AxisListType

B = 4096
C = 128
TWOB = 2 * B
N_CORES = 8
NB = 64                 # 128-row blocks in the full matrix
NBK = 39                # canonical key blocks each core loads (0..38)
KROWS = NBK * 128       # 4992
ISCALE = 2.0            # 1 / temperature
NA = 8                  # canonical stationary blocks per core (a = 0..7)
SPAN = 2048             # 16 key tiles per exp span; 2 spans per a
CHUNKS = [(0, 8), (8, 8), (16, 8), (24, 8), (32, 7)]  # keys DMA chunks


def _patch_act_tables():
    """Keep Exp/Ln only in natural_log_exp_and_others so bacc emits ONE
    ACT_TABLE_LOAD for the whole kernel."""
    if getattr(bacc, "_ntx_act_patched", False):
        return
    orig = bacc.get_activation_tables

    def patched(arch):
        out = {}
        for name, fns in orig(arch).items():
            if name != "natural_log_exp_and_others":
                fns = fns - {AF.Exp, AF.Ln}
            out[name] = fns
        return out

    bacc.get_activation_tables = patched
    bacc._ntx_act_patched = True


def build_bass() -> bass.Bass:
    _patch_act_tables()
    nc = bacc.Bacc()
    keys = nc.dram_tensor("keys", [KROWS, C], F32, kind="ExternalInput")
    q32 = nc.dram_tensor("q32", [512, C], F32, kind="ExternalInput")
    p32 = nc.dram_tensor("p32", [512, C], F32, kind="ExternalInput")
    cs_o = nc.dram_tensor("cs_o", [128, NBK - 1], F32, kind="ExternalOutput")
    rs_o = nc.dram_tensor("rs_o", [128, 3 * NA], F32, kind="ExternalOutput")
    d32_o = nc.dram_tensor("d32_o", [128, 12], F32, kind="ExternalOutput")

    with TileContext(nc) as tc:
        with (
            tc.tile_pool(name="big", bufs=1) as big,
            tc.tile_pool(name="small", bufs=1) as small,
            tc.tile_pool(name="scr", bufs=2) as scr,
            tc.tile_pool(name="expp", bufs=4) as expp,
            tc.tile_pool(name="ps", bufs=2, space="PSUM") as psp,
        ):
            # ---- identity/ones setup first (gpsimd), then key DMAs
            ident = small.tile([128, 128], F32)
            make_identity(nc, ident[:])
            identb = small.tile([128, 128], BF16)
            nc.vector.tensor_copy(identb[:], ident[:])
            onesb = small.tile([128, 1], BF16)
            nc.vector.memset(onesb[:], 1.0)

            # ---- input DMAs on sync+gpsimd rings (never ScalarE)
            kt3 = big.tile([128, NBK, C], F32)
            dma_engs = [nc.sync, nc.gpsimd, nc.sync, nc.gpsimd, nc.sync]
            for g, (b0, nt) in enumerate(CHUNKS):
                dma_engs[g].dma_start(
                    out=kt3[:, b0 : b0 + nt, :],
                    in_=keys[b0 * 128 : (b0 + nt) * 128, :].rearrange(
                        "(t p) c -> p t c", p=128
                    ),
                )
            q32t = small.tile([128, 4, C], F32)
            p32t = small.tile([128, 4, C], F32)

            # cs accumulator strip: canonical key cols 1..38
            A = big.tile([128, (NBK - 1) * 128], BF16)
            nc.gpsimd.memset(A[:], 0.0)

            # ---- norms: nrm[:, 0:39] keys, 39:43 q32, 43:47 p32
            nrm = small.tile([128, 48], F32)
            inv = small.tile([128, 48], F32)

            def norms(x3, col, n):
                sq = scr.tile([128, 16, C], F32, tag="sq")
                nc.vector.tensor_mul(sq[:, 0:n, :], x3[:], x3[:])
                nc.vector.reduce_sum(nrm[:, col : col + n], sq[:, 0:n, :], axis=AX.X)

            def rsqrt_batch(col, n):
                nc.scalar.activation(nrm[:, col : col + n], nrm[:, col : col + n], AF.Ln)
                nc.scalar.activation(
                    inv[:, col : col + n], nrm[:, col : col + n], AF.Exp, scale=-0.5
                )

            keysT = big.tile([128, KROWS], BF16)
            kn3 = big.tile([128, NBK, C], BF16)

            def transpose_scale(g):
                b0, nt = CHUNKS[g]
                nc.vector.tensor_mul(
                    kn3[:, b0 : b0 + nt, :],
                    kt3[:, b0 : b0 + nt, :],
                    inv[:, b0 : b0 + nt].unsqueeze(2).broadcast_to((128, nt, C)),
                )
                tq = psp.tile([128, 1024], BF16, tag="ps")
                for i in range(nt):
                    nc.tensor.transpose(
                        tq[:, i * 128 : (i + 1) * 128], kn3[:, b0 + i, :], identb[:]
                    )
                nc.vector.tensor_copy(
                    keysT[:, b0 * 128 : (b0 + nt) * 128], tq[:, 0 : nt * 128]
                )

            # head-critical: chunks 0+1 first
            norms(kt3[:, 0:16, :], 0, 16)
            rsqrt_batch(0, 16)
            transpose_scale(0)
            transpose_scale(1)

            rs = small.tile([128, 3 * NA], F32)

            # spans per a: p0 = d 0..7 (1024, incl diag), p1 = d 8..23
            # (2048), p2 = d 24..31 (1024).  p0 only needs key chunks 0-1.
            PARTS = [(0, 8), (8, 16), (24, 8)]

            def span(a, p):
                d0, nt = PARTS[p]
                c0 = (a + d0) * 128
                w = nt * 128
                psm = psp.tile([128, w], F32, tag="ps")
                lhs = keysT[:, a * 128 : (a + 1) * 128]
                for j in range(w // 512):
                    nc.tensor.matmul(
                        psm[:, j * 512 : (j + 1) * 512],
                        lhsT=lhs,
                        rhs=keysT[:, c0 + j * 512 : c0 + (j + 1) * 512],
                        start=True, stop=True,
                    )
                expb = expp.tile([128, w], BF16, tag="eb")
                nc.scalar.activation(
                    expb[:], psm[:], AF.Exp, scale=ISCALE,
                    accum_out=rs[:, 3 * a + p : 3 * a + p + 1],
                )
                # A cols (a+d0)..(a+d0+nt-1), diag tile excluded for p0
                if p == 0:
                    nc.vector.tensor_add(
                        A[:, a * 128 : (a + 7) * 128],
                        A[:, a * 128 : (a + 7) * 128],
                        expb[:, 128:1024],
                    )
                else:
                    lo = (a + d0 - 1) * 128
                    nc.vector.tensor_add(
                        A[:, lo : lo + w], A[:, lo : lo + w], expb[:]
                    )

            d32out = small.tile([128, 12], F32)
            exp32 = small.tile([128, 512], BF16)

            def d32_dma():
                nc.gpsimd.dma_start(
                    out=q32t[:], in_=q32[:].rearrange("(t p) c -> p t c", p=128)
                )
                nc.sync.dma_start(
                    out=p32t[:], in_=p32[:].rearrange("(t p) c -> p t c", p=128)
                )

            def d32_prep():
                norms(q32t, 39, 4)
                norms(p32t, 43, 4)
                rsqrt_batch(39, 8)
                q32n = scr.tile([128, 4, C], BF16, tag="d32n")
                p32n = scr.tile([128, 4, C], BF16, tag="d32n")
                nc.vector.tensor_mul(
                    q32n[:], q32t[:],
                    inv[:, 39:43].unsqueeze(2).broadcast_to((128, 4, C)),
                )
                nc.vector.tensor_mul(
                    p32n[:], p32t[:],
                    inv[:, 43:47].unsqueeze(2).broadcast_to((128, 4, C)),
                )
                tp = psp.tile([128, 1024], BF16, tag="ps")
                for t in range(4):
                    nc.tensor.transpose(
                        tp[:, t * 128 : (t + 1) * 128], q32n[:, t, :], identb[:]
                    )
                for t in range(4):
                    nc.tensor.transpose(
                        tp[:, 512 + t * 128 : 512 + (t + 1) * 128], p32n[:, t, :],
                        identb[:],
                    )
                qpT = small.tile([128, 1024], BF16)
                nc.vector.tensor_copy(qpT[:], tp[:])
                return qpT

            def d32_main(qpT):
                psm32 = psp.tile([128, 512], F32, tag="ps")
                for t in range(4):
                    nc.tensor.matmul(
                        psm32[:, t * 128 : (t + 1) * 128],
                        lhsT=qpT[:, t * 128 : (t + 1) * 128],
                        rhs=qpT[:, 512 + t * 128 : 512 + (t + 1) * 128],
                        start=True, stop=True,
                    )
                # pos32 = diagonals of the 4 normalized sim tiles
                dsc = scr.tile([128, 4, 128], F32, tag="dsc")
                nc.vector.tensor_mul(
                    dsc[:],
                    psm32[:].rearrange("p (t f) -> p t f", t=4),
                    ident[:].unsqueeze(1).broadcast_to((128, 4, 128)),
                )
                nc.vector.reduce_sum(d32out[:, 8:12], dsc[:], axis=AX.X)
                nc.scalar.activation(exp32[:], psm32[:], AF.Exp, scale=ISCALE)
                # rs32: free-axis sums on DVE
                nc.vector.reduce_sum(
                    d32out[:, 0:4],
                    exp32[:].rearrange("p (t f) -> p t f", t=4),
                    axis=AX.X,
                )
                # cs32: partition sums via PE fold
                csp32 = psp.tile([128, 4], F32, tag="ps")
                for t in range(4):
                    nc.tensor.matmul(
                        csp32[:, t : t + 1],
                        lhsT=exp32[:, t * 128 : (t + 1) * 128],
                        rhs=onesb[:],
                        start=True, stop=True,
                    )
                nc.vector.tensor_copy(d32out[:, 4:8], csp32[:])

            # ---- progressive A folds (col c of A final after its last
            # contributing span): cols 0-7 after wave1, 8-22 after wave2,
            # 23-37 after wave3.
            cs_sb = small.tile([128, NBK - 1], F32)

            def fold(c_lo, c_hi):
                csp = psp.tile([128, c_hi - c_lo], F32, tag="ps")
                for c in range(c_lo, c_hi):
                    nc.tensor.matmul(
                        csp[:, c - c_lo : c - c_lo + 1],
                        lhsT=A[:, c * 128 : (c + 1) * 128],
                        rhs=onesb[:],
                        start=True, stop=True,
                    )
                nc.vector.tensor_copy(cs_sb[:, c_lo:c_hi], csp[:])

            # ---- main: wave1 = p0 spans (chunks 0-1 only), then p1, p2
            span(0, 0)
            span(1, 0)
            norms(kt3[:, 16:32, :], 16, 16)
            span(2, 0)
            span(3, 0)
            rsqrt_batch(16, 16)
            d32_dma()
            span(4, 0)
            transpose_scale(2)
            span(5, 0)
            transpose_scale(3)
            span(6, 0)
            norms(kt3[:, 32:39, :], 32, 7)
            span(7, 0)
            rsqrt_batch(32, 7)
            fold(0, 8)
            span(0, 1)
            transpose_scale(4)
            span(1, 1)
            qpT = d32_prep()
            span(2, 1)
            span(3, 1)
            span(4, 1)
            d32_main(qpT)
            span(5, 1)
            span(6, 1)
            span(7, 1)
            fold(8, 23)
            span(0, 2)
            span(1, 2)
            span(2, 2)
            span(3, 2)
            span(4, 2)
            span(5, 2)
            span(6, 2)
            span(7, 2)
            fold(23, NBK - 1)
            nc.sync.dma_start(out=cs_o[:], in_=cs_sb[:])
            nc.gpsimd.dma_start(out=rs_o[:], in_=rs[:])
            nc.sync.dma_start(out=d32_o[:], in_=d32out[:])

    nc.finalize()
    return nc


_NC_CACHE: bass.Bass | None = None
LAST_RESULTS = None  # BassKernelResults of the last run (for profiling)


def _get_nc() -> bass.Bass:
    global _NC_CACHE
    if _NC_CACHE is None:
        _NC_CACHE = build_bass()
    return _NC_CACHE


def kernel(z_i: np.ndarray, z_j: np.ndarray) -> np.ndarray:
    global LAST_RESULTS
    z_i = np.asarray(z_i, dtype=np.float32)
    z_j = np.asarray(z_j, dtype=np.float32)
    assert z_i.shape == (B, C) and z_j.shape == (B, C)

    reps = np.concatenate([z_i, z_j], axis=0)  # [2B, C]
    reps64 = reps.reshape(NB, 128, C)
    in_maps = []
    for k in range(N_CORES):
        rot = np.roll(reps64, -8 * k, axis=0)
        in_maps.append(
            {
                "keys": np.ascontiguousarray(rot[:NBK].reshape(KROWS, C)),
                "q32": np.ascontiguousarray(
                    reps64[4 * k : 4 * k + 4].reshape(512, C)
                ),
                "p32": np.ascontiguousarray(
                    reps64[4 * k + 32 : 4 * k + 36].reshape(512, C)
                ),
            }
        )

    nc = _get_nc()
    trace = bool(int(os.environ.get("KERNEL_TRACE", "0")))
    res = run_bass_kernel_spmd(
        nc, in_maps, core_ids=list(range(N_CORES)), trace=trace
    )
    LAST_RESULTS = res

    # ---- host assembly (tiny: 8192-row scatter + ln in fp64)
    S64 = np.zeros((NB, 128), dtype=np.float64)
    pos = np.zeros((NB, 128), dtype=np.float64)
    for k in range(N_CORES):
        r = res.results[k]
        cs = np.asarray(r["cs_o"], dtype=np.float64)      # [128, 38]
        rs = np.asarray(r["rs_o"], dtype=np.float64)      # [128, 24]
        d32 = np.asarray(r["d32_o"], dtype=np.float64)    # [128, 12]
        ccols = (np.arange(1, NBK) + 8 * k) % NB          # 38 global blocks
        np.add.at(S64, ccols, cs.T)
        acols = (np.arange(NA) + 8 * k) % NB              # 8 global blocks
        np.add.at(S64, acols, rs.reshape(128, NA, 3).sum(-1).T)
        a32 = 4 * k + np.arange(4)
        b32 = a32 + 32
        np.add.at(S64, a32, d32[:, 0:4].T)                # rs32
        np.add.at(S64, b32, d32[:, 4:8].T)                # cs32
        pos[a32] = d32[:, 8:12].T
        pos[b32] = d32[:, 8:12].T

    Sv = S64.reshape(TWOB)
    pv = pos.reshape(TWOB)
    tot = Sv - np.exp(2.0) + np.exp(2.0 * pv)
    loss = np.mean(np.log(tot) - 2.0 * pv)
    return np.float32(loss)


# revision 23
# speedup vs baseline: 1.0209x; 1.0209x over previous
"""NT-Xent loss kernel for Trainium2, 8-core SPMD — v6 (symmetric triangle).

Math (matches the reference):
  reps = concat(z_i, z_j)  [2B, C], B=4096, C=128; rhat = reps/|reps|
  S_r = sum_c exp(2 rhat_r . rhat_c);  pos_r = rhat_r . rhat_{(r+B)%2B}
  loss = mean_r( ln(S_r - e^2 + e^{2 pos_r}) - 2 pos_r )

exp(2 sim) is symmetric, so each unordered pair is exp'd ONCE:
  Rotation decomposition of the 64x64 block grid (blocks of 128 rows):
  every core runs the SAME canonical program computing tiles (a, a+d),
  a in 0..7, d in 0..31, on keys ROTATED by 8k blocks (host np.roll).
  Across 8 cores this covers every pair with |d| in 0..31 exactly once.
  The d=32 pairs (also the positive pairs) don't decompose under the
  rotation group; they are fed as separate per-core gathered inputs
  q32/p32 (4 tiles per core), whose diagonals give pos_r for free.

Per tile both sums are extracted: row sums ride ScalarE's accum_out;
column sums are accumulated elementwise into a bf16 strip A (DVE adds)
and partition-folded ONCE at the end on the PE (per-tile lhsT=A-tile
matmul against a ones vector). Host scatter-adds the per-core partials
(static maps), subtracts e^2 for the diagonal, and takes ln/mean in
fp64 over the 8192 rows.

Per core: 260 exp tiles (vs 512 full-matrix), 2.44MB keys DMA (only
blocks 0..38 are referenced canonically), bf16 matmuls (1024-wide
moving operand), normalization folded into the transposed copy via a
partition-broadcast row of 1/|k| (one DVE pass).
"""

import os

import numpy as np

import concourse.bacc as bacc
import concourse.bass as bass
import concourse.mybir as mybir
from concourse.bass_utils import run_bass_kernel_spmd
from concourse.masks import make_identity
from concourse.tile import TileContext

F32 = mybir.dt.float32
F32R = mybir.dt.float32r
BF16 = mybir.dt.bfloat16
AF = mybir.ActivationFunctionType
ALU = mybir.AluOpType
AX = mybir.AxisListType

B = 4096
C = 128
TWOB = 2 * B
N_CORES = 8
NB = 64                 # 128-row blocks in the full matrix
NBK = 39                # canonical key blocks each core loads (0..38)
KROWS = NBK * 128       # 4992
ISCALE = 2.0            # 1 / temperature
NA = 8                  # canonical stationary blocks per core (a = 0..7)
SPAN = 2048             # 16 key tiles per exp span; 2 spans per a
CHUNKS = [(0, 8), (8, 8), (16, 8), (24, 8), (32, 7)]  # keys DMA chunks


def _patch_act_tables():
    """Keep Exp/Ln only in natural_log_exp_and_others so bacc emits ONE
    ACT_TABLE_LOAD for the whole kernel."""
    if getattr(bacc, "_ntx_act_patched", False):
        return
    orig = bacc.get_activation_tables

    def patched(arch):
        out = {}
        for name, fns in orig(arch).items():
            if name != "natural_log_exp_and_others":
                fns = fns - {AF.Exp, AF.Ln}
            out[name] = fns
        return out

    bacc.get_activation_tables = patched
    bacc._ntx_act_patched = True


def build_bass() -> bass.Bass:
    _patch_act_tables()
    nc = bacc.Bacc()
    keys = nc.dram_tensor("keys", [KROWS, C], F32, kind="ExternalInput")
    q32 = nc.dram_tensor("q32", [512, C], F32, kind="ExternalInput")
    p32 = nc.dram_tensor("p32", [512, C], F32, kind="ExternalInput")
    cs_o = nc.dram_tensor("cs_o", [128, NBK - 1], F32, kind="ExternalOutput")
    rs_o = nc.dram_tensor("rs_o", [128, 3 * NA], F32, kind="ExternalOutput")
    d32_o = nc.dram_tensor("d32_o", [128, 12], F32, kind="ExternalOutput")

    with TileContext(nc) as tc:
        with (
            tc.tile_pool(name="big", bufs=1) as big,
            tc.tile_pool(name="small", bufs=1) as small,
            tc.tile_pool(name="scr", bufs=2) as scr,
            tc.tile_pool(name="expp", bufs=4) as expp,
            tc.tile_pool(name="ps", bufs=2, space="PSUM") as psp,
        ):
            # ---- identity/ones setup first (gpsimd), then key DMAs
            ident = small.tile([128, 128], F32)
            make_identity(nc, ident[:])
            identb = small.tile([128, 128], BF16)
            nc.vector.tensor_copy(identb[:], ident[:])
            onesb = small.tile([128, 1], BF16)
            nc.vector.memset(onesb[:], 1.0)

            # ---- input DMAs on sync+gpsimd rings (never ScalarE)
            kt3 = big.tile([128, NBK, C], F32)
            dma_engs = [nc.sync, nc.gpsimd, nc.sync, nc.gpsimd, nc.sync]
            for g, (b0, nt) in enumerate(CHUNKS):
                dma_engs[g].dma_start(
                    out=kt3[:, b0 : b0 + nt, :],
                    in_=keys[b0 * 128 : (b0 + nt) * 128, :].rearrange(
                        "(t p) c -> p t c", p=128
                    ),
                )
            q32t = small.tile([128, 4, C], F32)
            p32t = small.tile([128, 4, C], F32)

            # cs accumulator strip: canonical key cols 1..38
            A = big.tile([128, (NBK - 1) * 128], BF16)
            nc.gpsimd.memset(A[:], 0.0)

            # ---- norms: nrm[:, 0:39] keys, 39:43 q32, 43:47 p32
            nrm = small.tile([128, 48], F32)
            inv = small.tile([128, 48], F32)

            def norms(x3, col, n):
                sq = scr.tile([128, 16, C], F32, tag="sq")
                nc.vector.tensor_mul(sq[:, 0:n, :], x3[:], x3[:])
                nc.vector.reduce_sum(nrm[:, col : col + n], sq[:, 0:n, :], axis=AX.X)

            def rsqrt_batch(col, n):
                nc.scalar.activation(nrm[:, col : col + n], nrm[:, col : col + n], AF.Ln)
                nc.scalar.activation(
                    inv[:, col : col + n], nrm[:, col : col + n], AF.Exp, scale=-0.5
                )

            keysT = big.tile([128, KROWS], BF16)
            kn3 = big.tile([128, NBK, C], BF16)

            def transpose_scale(g):
                b0, nt = CHUNKS[g]
                nc.vector.tensor_mul(
                    kn3[:, b0 : b0 + nt, :],
                    kt3[:, b0 : b0 + nt, :],
                    inv[:, b0 : b0 + nt].unsqueeze(2).broadcast_to((128, nt, C)),
                )
                tq = psp.tile([128, 1024], BF16, tag="ps")
                for i in range(nt):
                    nc.tensor.transpose(
                        tq[:, i * 128 : (i + 1) * 128], kn3[:, b0 + i, :], identb[:]
                    )
                nc.vector.tensor_copy(
                    keysT[:, b0 * 128 : (b0 + nt) * 128], tq[:, 0 : nt * 128]
                )

            # head-critical: chunks 0+1 first
            norms(kt3[:, 0:16, :], 0, 16)
            rsqrt_batch(0, 16)
            transpose_scale(0)
            transpose_scale(1)

            rs = small.tile([128, 3 * NA], F32)

            # spans per a: p0 = d 0..7 (1024, incl diag), p1 = d 8..23
            # (2048), p2 = d 24..31 (1024).  p0 only needs key chunks 0-1.
            PARTS = [(0, 8), (8, 16), (24, 8)]

            def span(a, p):
                d0, nt = PARTS[p]
                c0 = (a + d0) * 128
                w = nt * 128
                psm = psp.tile([128, w], F32, tag="ps")
                lhs = keysT[:, a * 128 : (a + 1) * 128]
                for j in range(w // 512):
                    nc.tensor.matmul(
                        psm[:, j * 512 : (j + 1) * 512],
                        lhsT=lhs,
                        rhs=keysT[:, c0 + j * 512 : c0 + (j + 1) * 512],
                        start=True, stop=True,
                    )
                expb = expp.tile([128, w], BF16, tag="eb")
                nc.scalar.activation(
                    expb[:], psm[:], AF.Exp, scale=ISCALE,
                    accum_out=rs[:, 3 * a + p : 3 * a + p + 1],
                )
                # A cols (a+d0)..(a+d0+nt-1), diag tile excluded for p0
                if p == 0:
                    nc.vector.tensor_add(
                        A[:, a * 128 : (a + 7) * 128],
                        A[:, a * 128 : (a + 7) * 128],
                        expb[:, 128:1024],
                    )
                else:
                    lo = (a + d0 - 1) * 128
                    nc.vector.tensor_add(
                        A[:, lo : lo + w], A[:, lo : lo + w], expb[:]
                    )

            d32out = small.tile([128, 12], F32)
            exp32 = small.tile([128, 512], BF16)

            def d32_dma():
                nc.gpsimd.dma_start(
                    out=q32t[:], in_=q32[:].rearrange("(t p) c -> p t c", p=128)
                )
                nc.sync.dma_start(
                    out=p32t[:], in_=p32[:].rearrange("(t p) c -> p t c", p=128)
                )

            def d32_prep():
                norms(q32t, 39, 4)
                norms(p32t, 43, 4)
                rsqrt_batch(39, 8)
                q32n = scr.tile([128, 4, C], BF16, tag="d32n")
                p32n = scr.tile([128, 4, C], BF16, tag="d32n")
                nc.vector.tensor_mul(
                    q32n[:], q32t[:],
                    inv[:, 39:43].unsqueeze(2).broadcast_to((128, 4, C)),
                )
                nc.vector.tensor_mul(
                    p32n[:], p32t[:],
                    inv[:, 43:47].unsqueeze(2).broadcast_to((128, 4, C)),
                )
                tp = psp.tile([128, 1024], BF16, tag="ps")
                for t in range(4):
                    nc.tensor.transpose(
                        tp[:, t * 128 : (t + 1) * 128], q32n[:, t, :], identb[:]
                    )
                for t in range(4):
                    nc.tensor.transpose(
                        tp[:, 512 + t * 128 : 512 + (t + 1) * 128], p32n[:, t, :],
                        identb[:],
                    )
                qpT = small.tile([128, 1024], BF16)
                nc.vector.tensor_copy(qpT[:], tp[:])
                return qpT

            def d32_main(qpT):
                psm32 = psp.tile([128, 512], F32, tag="ps")
                for t in range(4):
                    nc.tensor.matmul(
                        psm32[:, t * 128 : (t + 1) * 128],
                        lhsT=qpT[:, t * 128 : (t + 1) * 128],
                        rhs=qpT[:, 512 + t * 128 : 512 + (t + 1) * 128],
                        start=True, stop=True,
                    )
                # pos32 = diagonals of the 4 normalized sim tiles
                dsc = scr.tile([128, 4, 128], F32, tag="dsc")
                nc.vector.tensor_mul(
                    dsc[:],
                    psm32[:].rearrange("p (t f) -> p t f", t=4),
                    ident[:].unsqueeze(1).broadcast_to((128, 4, 128)),
                )
                nc.vector.reduce_sum(d32out[:, 8:12], dsc[:], axis=AX.X)
                nc.scalar.activation(exp32[:], psm32[:], AF.Exp, scale=ISCALE)
                # rs32: free-axis sums on DVE
                nc.vector.reduce_sum(
                    d32out[:, 0:4],
                    exp32[:].rearrange("p (t f) -> p t f", t=4),
                    axis=AX.X,
                )
                # cs32: partition sums via PE fold
                csp32 = psp.tile([128, 4], F32, tag="ps")
                for t in range(4):
                    nc.tensor.matmul(
                        csp32[:, t : t + 1],
                        lhsT=exp32[:, t * 128 : (t + 1) * 128],
                        rhs=onesb[:],
                        start=True, stop=True,
                    )
                nc.vector.tensor_copy(d32out[:, 4:8], csp32[:])

            # ---- progressive A folds (A-idx j final after its last
            # contributing span): p0 touches j in [a, a+6], p1 [a+7, a+22],
            # p2 [a+23, a+30] -> [0,7) final after wave1, [7,23) after
            # wave2, [23,38) after wave3.
            cs_sb = small.tile([128, NBK - 1], F32)

            def fold(c_lo, c_hi):
                csp = psp.tile([128, c_hi - c_lo], F32, tag="ps")
                for c in range(c_lo, c_hi):
                    nc.tensor.matmul(
                        csp[:, c - c_lo : c - c_lo + 1],
                        lhsT=A[:, c * 128 : (c + 1) * 128],
                        rhs=onesb[:],
                        start=True, stop=True,
                    )
                nc.vector.tensor_copy(cs_sb[:, c_lo:c_hi], csp[:])

            # ---- main: wave1 = p0 spans (chunks 0-1 only), then p1, p2
            span(0, 0)
            span(1, 0)
            norms(kt3[:, 16:32, :], 16, 16)
            span(2, 0)
            span(3, 0)
            rsqrt_batch(16, 16)
            d32_dma()
            span(4, 0)
            transpose_scale(2)
            span(5, 0)
            transpose_scale(3)
            span(6, 0)
            norms(kt3[:, 32:39, :], 32, 7)
            span(7, 0)
            rsqrt_batch(32, 7)
            fold(0, 7)
            span(0, 1)
            transpose_scale(4)
            span(1, 1)
            qpT = d32_prep()
            span(2, 1)
            span(3, 1)
            span(4, 1)
            d32_main(qpT)
            span(5, 1)
            span(6, 1)
            span(7, 1)
            fold(7, 23)
            span(0, 2)
            span(1, 2)
            span(2, 2)
            span(3, 2)
            span(4, 2)
            span(5, 2)
            span(6, 2)
            span(7, 2)
            fold(23, NBK - 1)
            nc.sync.dma_start(out=cs_o[:], in_=cs_sb[:])
            nc.gpsimd.dma_start(out=rs_o[:], in_=rs[:])
            nc.sync.dma_start(out=d32_o[:], in_=d32out[:])

    nc.finalize()
    return nc


_NC_CACHE: bass.Bass | None = None
LAST_RESULTS = None  # BassKernelResults of the last run (for profiling)


def _get_nc() -> bass.Bass:
    global _NC_CACHE
    if _NC_CACHE is None:
        _NC_CACHE = build_bass()
    return _NC_CACHE


def kernel(z_i: np.ndarray, z_j: np.ndarray) -> np.ndarray:
    global LAST_RESULTS
    z_i = np.asarray(z_i, dtype=np.float32)
    z_j = np.asarray(z_j, dtype=np.float32)
    assert z_i.shape == (B, C) and z_j.shape == (B, C)

    reps = np.concatenate([z_i, z_j], axis=0)  # [2B, C]
    reps64 = reps.reshape(NB, 128, C)
    in_maps = []
    for k in range(N_CORES):
        rot = np.roll(reps64, -8 * k, axis=0)
        in_maps.append(
            {
                "keys": np.ascontiguousarray(rot[:NBK].reshape(KROWS, C)),
                "q32": np.ascontiguousarray(
                    reps64[4 * k : 4 * k + 4].reshape(512, C)
                ),
                "p32": np.ascontiguousarray(
                    reps64[4 * k + 32 : 4 * k + 36].reshape(512, C)
                ),
            }
        )

    nc = _get_nc()
    trace = bool(int(os.environ.get("KERNEL_TRACE", "0")))
    res = run_bass_kernel_spmd(
        nc, in_maps, core_ids=list(range(N_CORES)), trace=trace
    )
    LAST_RESULTS = res

    # ---- host assembly (tiny: 8192-row scatter + ln in fp64)
    S64 = np.zeros((NB, 128), dtype=np.float64)
    pos = np.zeros((NB, 128), dtype=np.float64)
    for k in range(N_CORES):
        r = res.results[k]
        cs = np.asarray(r["cs_o"], dtype=np.float64)      # [128, 38]
        rs = np.asarray(r["rs_o"], dtype=np.float64)      # [128, 24]
        d32 = np.asarray(r["d32_o"], dtype=np.float64)    # [128, 12]
        ccols = (np.arange(1, NBK) + 8 * k) % NB          # 38 global blocks
        np.add.at(S64, ccols, cs.T)
        acols = (np.arange(NA) + 8 * k) % NB              # 8 global blocks
        np.add.at(S64, acols, rs.reshape(128, NA, 3).sum(-1).T)
        a32 = 4 * k + np.arange(4)
        b32 = a32 + 32
        np.add.at(S64, a32, d32[:, 0:4].T)                # rs32
        np.add.at(S64, b32, d32[:, 4:8].T)                # cs32
        pos[a32] = d32[:, 8:12].T
        pos[b32] = d32[:, 8:12].T

    Sv = S64.reshape(TWOB)
    pv = pos.reshape(TWOB)
    tot = Sv - np.exp(2.0) + np.exp(2.0 * pv)
    loss = np.mean(np.log(tot) - 2.0 * pv)
    return np.float32(loss)


# revision 28
# speedup vs baseline: 1.2175x; 1.1925x over previous
"""NT-Xent loss kernel for Trainium2, 8-core SPMD — v6 (symmetric triangle).

Math (matches the reference):
  reps = concat(z_i, z_j)  [2B, C], B=4096, C=128; rhat = reps/|reps|
  S_r = sum_c exp(2 rhat_r . rhat_c);  pos_r = rhat_r . rhat_{(r+B)%2B}
  loss = mean_r( ln(S_r - e^2 + e^{2 pos_r}) - 2 pos_r )

exp(2 sim) is symmetric, so each unordered pair is exp'd ONCE:
  Rotation decomposition of the 64x64 block grid (blocks of 128 rows):
  every core runs the SAME canonical program computing tiles (a, a+d),
  a in 0..7, d in 0..31, on keys ROTATED by 8k blocks (host np.roll).
  Across 8 cores this covers every pair with |d| in 0..31 exactly once.
  The d=32 pairs (also the positive pairs) don't decompose under the
  rotation group; they are fed as separate per-core gathered inputs
  q32/p32 (4 tiles per core), whose diagonals give pos_r for free.

Per tile both sums are extracted: row sums ride ScalarE's accum_out;
column sums are accumulated elementwise into a bf16 strip A (DVE adds)
and partition-folded ONCE at the end on the PE (per-tile lhsT=A-tile
matmul against a ones vector). Host scatter-adds the per-core partials
(static maps), subtracts e^2 for the diagonal, and takes ln/mean in
fp64 over the 8192 rows.

Per core: 260 exp tiles (vs 512 full-matrix), 2.44MB keys DMA (only
blocks 0..38 are referenced canonically), bf16 matmuls (1024-wide
moving operand), normalization folded into the transposed copy via a
partition-broadcast row of 1/|k| (one DVE pass).
"""

import os

import numpy as np

import concourse.bacc as bacc
import concourse.bass as bass
import concourse.mybir as mybir
from concourse.bass_utils import run_bass_kernel_spmd
from concourse.masks import make_identity
from concourse.tile import TileContext

F32 = mybir.dt.float32
F32R = mybir.dt.float32r
BF16 = mybir.dt.bfloat16
AF = mybir.ActivationFunctionType
ALU = mybir.AluOpType
AX = mybir.AxisListType

B = 4096
C = 128
TWOB = 2 * B
N_CORES = 8
NB = 64                 # 128-row blocks in the full matrix
NBK = 39                # canonical key blocks each core loads (0..38)
KROWS = NBK * 128       # 4992
ISCALE = 2.0            # 1 / temperature
NA = 8                  # canonical stationary blocks per core (a = 0..7)
# keys DMA in 10 half-chunks, alternating rings so completion is ordered
DMAS = [(4 * h, 4 if 4 * h + 4 <= NBK else NBK - 4 * h) for h in range(10)]
GROUPS = [(0, 8), (8, 8), (16, 8), (24, 8), (32, 7)]  # norm/transpose groups


def _patch_act_tables():
    """Keep Exp/Ln only in natural_log_exp_and_others so bacc emits ONE
    ACT_TABLE_LOAD for the whole kernel."""
    if getattr(bacc, "_ntx_act_patched", False):
        return
    orig = bacc.get_activation_tables

    def patched(arch):
        out = {}
        for name, fns in orig(arch).items():
            if name != "natural_log_exp_and_others":
                fns = fns - {AF.Exp, AF.Ln}
            out[name] = fns
        return out

    bacc.get_activation_tables = patched
    bacc._ntx_act_patched = True


def build_bass() -> bass.Bass:
    _patch_act_tables()
    nc = bacc.Bacc()
    keys = nc.dram_tensor("keys", [KROWS, C], F32, kind="ExternalInput")
    q32 = nc.dram_tensor("q32", [512, C], F32, kind="ExternalInput")
    p32 = nc.dram_tensor("p32", [512, C], F32, kind="ExternalInput")
    cs_o = nc.dram_tensor("cs_o", [128, NBK - 1], F32, kind="ExternalOutput")
    rs_o = nc.dram_tensor("rs_o", [128, 3 * NA], F32, kind="ExternalOutput")
    d32_o = nc.dram_tensor("d32_o", [128, 12], F32, kind="ExternalOutput")

    with TileContext(nc) as tc:
        with (
            tc.tile_pool(name="big", bufs=1) as big,
            tc.tile_pool(name="small", bufs=1) as small,
            tc.tile_pool(name="scr", bufs=2) as scr,
            tc.tile_pool(name="expp", bufs=4) as expp,
            tc.tile_pool(name="ps", bufs=2, space="PSUM") as psp,
            tc.tile_pool(name="psq", bufs=2, space="PSUM") as psq,
        ):
            # ---- identity/ones setup first (gpsimd), then key DMAs
            ident = small.tile([128, 128], F32)
            make_identity(nc, ident[:])
            identb = small.tile([128, 128], BF16)
            nc.vector.tensor_copy(identb[:], ident[:])
            onesb = small.tile([128, 1], BF16)
            nc.vector.memset(onesb[:], 1.0)

            # ---- input DMAs on sync+gpsimd rings (never ScalarE); half-
            # chunks alternate rings so chunks complete in order
            kt3 = big.tile([128, NBK, C], F32)
            for h, (b0, nt) in enumerate(DMAS):
                eng = nc.sync if h % 2 == 0 else nc.gpsimd
                eng.dma_start(
                    out=kt3[:, b0 : b0 + nt, :],
                    in_=keys[b0 * 128 : (b0 + nt) * 128, :].rearrange(
                        "(t p) c -> p t c", p=128
                    ),
                )
            q32t = small.tile([128, 4, C], F32)
            p32t = small.tile([128, 4, C], F32)

            # cs accumulator strip: canonical key cols 1..38
            A = big.tile([128, (NBK - 1) * 128], BF16)
            nc.gpsimd.memset(A[:], 0.0)

            # ---- norms: nrm[:, 0:39] keys, 39:43 q32, 43:47 p32
            nrm = small.tile([128, 48], F32)
            inv = small.tile([128, 48], F32)

            def norms(x3, col, n):
                sq = scr.tile([128, 16, C], F32, tag="sq")
                nc.vector.tensor_mul(sq[:, 0:n, :], x3[:], x3[:])
                nc.vector.reduce_sum(nrm[:, col : col + n], sq[:, 0:n, :], axis=AX.X)

            def rsqrt_batch(col, n):
                nc.scalar.activation(nrm[:, col : col + n], nrm[:, col : col + n], AF.Ln)
                nc.scalar.activation(
                    inv[:, col : col + n], nrm[:, col : col + n], AF.Exp, scale=-0.5
                )

            keysT = big.tile([128, KROWS], BF16)
            kn3 = big.tile([128, NBK, C], BF16)

            def transpose_scale(g):
                b0, nt = GROUPS[g]
                nc.vector.tensor_mul(
                    kn3[:, b0 : b0 + nt, :],
                    kt3[:, b0 : b0 + nt, :],
                    inv[:, b0 : b0 + nt].unsqueeze(2).broadcast_to((128, nt, C)),
                )
                tq = psq.tile([128, 1024], BF16, tag="pq")
                for i in range(nt):
                    nc.tensor.transpose(
                        tq[:, i * 128 : (i + 1) * 128], kn3[:, b0 + i, :], identb[:]
                    )
                nc.vector.tensor_copy(
                    keysT[:, b0 * 128 : (b0 + nt) * 128], tq[:, 0 : nt * 128]
                )

            # head-critical: group 0 (blocks 0-7) first
            norms(kt3[:, 0:8, :], 0, 8)
            rsqrt_batch(0, 8)
            transpose_scale(0)

            rs = small.tile([128, 3 * NA], F32)

            # spans per a: p0 = d 0..7 (1024, incl diag), p1 = d 8..19
            # (1536), p2 = d 20..31 (1536).  p0 only needs key group 0.
            PARTS = [(0, 8), (8, 12), (20, 12)]

            def span(a, p):
                d0, nt = PARTS[p]
                c0 = (a + d0) * 128
                w = nt * 128
                psm = psp.tile([128, w], F32, tag="ps")
                lhs = keysT[:, a * 128 : (a + 1) * 128]
                for j in range(w // 512):
                    nc.tensor.matmul(
                        psm[:, j * 512 : (j + 1) * 512],
                        lhsT=lhs,
                        rhs=keysT[:, c0 + j * 512 : c0 + (j + 1) * 512],
                        start=True, stop=True,
                    )
                expb = expp.tile([128, w], BF16, tag="eb")
                nc.scalar.activation(
                    expb[:], psm[:], AF.Exp, scale=ISCALE,
                    accum_out=rs[:, 3 * a + p : 3 * a + p + 1],
                )
                # A cols (a+d0)..(a+d0+nt-1), diag tile excluded for p0
                if p == 0:
                    nc.vector.tensor_add(
                        A[:, a * 128 : (a + 7) * 128],
                        A[:, a * 128 : (a + 7) * 128],
                        expb[:, 128:1024],
                    )
                else:
                    lo = (a + d0 - 1) * 128
                    nc.vector.tensor_add(
                        A[:, lo : lo + w], A[:, lo : lo + w], expb[:]
                    )

            d32out = small.tile([128, 12], F32)
            exp32 = small.tile([128, 512], BF16)

            def d32_dma():
                nc.gpsimd.dma_start(
                    out=q32t[:], in_=q32[:].rearrange("(t p) c -> p t c", p=128)
                )
                nc.sync.dma_start(
                    out=p32t[:], in_=p32[:].rearrange("(t p) c -> p t c", p=128)
                )

            def d32_prep():
                norms(q32t, 39, 4)
                norms(p32t, 43, 4)
                rsqrt_batch(39, 8)
                q32n = scr.tile([128, 4, C], BF16, tag="d32n")
                p32n = scr.tile([128, 4, C], BF16, tag="d32n")
                nc.vector.tensor_mul(
                    q32n[:], q32t[:],
                    inv[:, 39:43].unsqueeze(2).broadcast_to((128, 4, C)),
                )
                nc.vector.tensor_mul(
                    p32n[:], p32t[:],
                    inv[:, 43:47].unsqueeze(2).broadcast_to((128, 4, C)),
                )
                tp = psq.tile([128, 1024], BF16, tag="pq")
                for t in range(4):
                    nc.tensor.transpose(
                        tp[:, t * 128 : (t + 1) * 128], q32n[:, t, :], identb[:]
                    )
                for t in range(4):
                    nc.tensor.transpose(
                        tp[:, 512 + t * 128 : 512 + (t + 1) * 128], p32n[:, t, :],
                        identb[:],
                    )
                qpT = small.tile([128, 1024], BF16)
                nc.vector.tensor_copy(qpT[:], tp[:])
                return qpT

            def d32_main(qpT):
                psm32 = psq.tile([128, 512], F32, tag="pq")
                for t in range(4):
                    nc.tensor.matmul(
                        psm32[:, t * 128 : (t + 1) * 128],
                        lhsT=qpT[:, t * 128 : (t + 1) * 128],
                        rhs=qpT[:, 512 + t * 128 : 512 + (t + 1) * 128],
                        start=True, stop=True,
                    )
                # pos32 = diagonals of the 4 normalized sim tiles
                dsc = scr.tile([128, 4, 128], F32, tag="dsc")
                nc.vector.tensor_mul(
                    dsc[:],
                    psm32[:].rearrange("p (t f) -> p t f", t=4),
                    ident[:].unsqueeze(1).broadcast_to((128, 4, 128)),
                )
                nc.vector.reduce_sum(d32out[:, 8:12], dsc[:], axis=AX.X)
                nc.scalar.activation(exp32[:], psm32[:], AF.Exp, scale=ISCALE)
                # rs32: free-axis sums on DVE
                nc.vector.reduce_sum(
                    d32out[:, 0:4],
                    exp32[:].rearrange("p (t f) -> p t f", t=4),
                    axis=AX.X,
                )
                # cs32: partition sums via PE fold
                csp32 = psq.tile([128, 4], F32, tag="pq")
                for t in range(4):
                    nc.tensor.matmul(
                        csp32[:, t : t + 1],
                        lhsT=exp32[:, t * 128 : (t + 1) * 128],
                        rhs=onesb[:],
                        start=True, stop=True,
                    )
                nc.vector.tensor_copy(d32out[:, 4:8], csp32[:])

            # ---- progressive A folds (A-idx j final after its last
            # contributing span): p0 touches j in [a, a+6], p1 [a+7, a+18],
            # p2 [a+19, a+30].
            cs_sb = small.tile([128, NBK - 1], F32)

            def fold(c_lo, c_hi):
                csp = psq.tile([128, c_hi - c_lo], F32, tag="pq")
                for c in range(c_lo, c_hi):
                    nc.tensor.matmul(
                        csp[:, c - c_lo : c - c_lo + 1],
                        lhsT=A[:, c * 128 : (c + 1) * 128],
                        rhs=onesb[:],
                        start=True, stop=True,
                    )
                nc.vector.tensor_copy(cs_sb[:, c_lo:c_hi], csp[:])

            # ---- main: per-group prep feeds wave1 (p0, group 0-1
            # only), then wave2 (p1), wave3 (p2); folds as cols finalize
            norms(kt3[:, 8:16, :], 8, 8)
            rsqrt_batch(8, 8)
            transpose_scale(1)
            span(0, 0)
            span(1, 0)
            norms(kt3[:, 16:24, :], 16, 8)
            span(2, 0)
            rsqrt_batch(16, 8)
            span(3, 0)
            transpose_scale(2)
            d32_dma()
            span(4, 0)
            norms(kt3[:, 24:32, :], 24, 8)
            span(5, 0)
            rsqrt_batch(24, 8)
            span(6, 0)
            transpose_scale(3)
            span(7, 0)
            fold(0, 7)
            norms(kt3[:, 32:39, :], 32, 7)
            rsqrt_batch(32, 7)
            transpose_scale(4)
            span(0, 1)
            span(1, 1)
            qpT = d32_prep()
            span(2, 1)
            span(3, 1)
            span(4, 1)
            d32_main(qpT)
            nc.gpsimd.dma_start(out=d32_o[:], in_=d32out[:])
            span(5, 1)
            span(6, 1)
            span(7, 1)
            fold(7, 19)
            span(0, 2)
            span(1, 2)
            span(2, 2)
            span(3, 2)
            span(4, 2)
            span(5, 2)
            span(6, 2)
            fold(19, 26)
            span(7, 2)
            nc.gpsimd.dma_start(out=rs_o[:], in_=rs[:])
            fold(26, NBK - 1)
            nc.sync.dma_start(out=cs_o[:], in_=cs_sb[:])

    nc.finalize()
    return nc


_NC_CACHE: bass.Bass | None = None
LAST_RESULTS = None  # BassKernelResults of the last run (for profiling)


def _get_nc() -> bass.Bass:
    global _NC_CACHE
    if _NC_CACHE is None:
        _NC_CACHE = build_bass()
    return _NC_CACHE


def kernel(z_i: np.ndarray, z_j: np.ndarray) -> np.ndarray:
    global LAST_RESULTS
    z_i = np.asarray(z_i, dtype=np.float32)
    z_j = np.asarray(z_j, dtype=np.float32)
    assert z_i.shape == (B, C) and z_j.shape == (B, C)

    reps = np.concatenate([z_i, z_j], axis=0)  # [2B, C]
    reps64 = reps.reshape(NB, 128, C)
    in_maps = []
    for k in range(N_CORES):
        rot = np.roll(reps64, -8 * k, axis=0)
        in_maps.append(
            {
                "keys": np.ascontiguousarray(rot[:NBK].reshape(KROWS, C)),
                "q32": np.ascontiguousarray(
                    reps64[4 * k : 4 * k + 4].reshape(512, C)
                ),
                "p32": np.ascontiguousarray(
                    reps64[4 * k + 32 : 4 * k + 36].reshape(512, C)
                ),
            }
        )

    nc = _get_nc()
    trace = bool(int(os.environ.get("KERNEL_TRACE", "0")))
    res = run_bass_kernel_spmd(
        nc, in_maps, core_ids=list(range(N_CORES)), trace=trace
    )
    LAST_RESULTS = res

    # ---- host assembly (tiny: 8192-row scatter + ln in fp64)
    S64 = np.zeros((NB, 128), dtype=np.float64)
    pos = np.zeros((NB, 128), dtype=np.float64)
    for k in range(N_CORES):
        r = res.results[k]
        cs = np.asarray(r["cs_o"], dtype=np.float64)      # [128, 38]
        rs = np.asarray(r["rs_o"], dtype=np.float64)      # [128, 24]
        d32 = np.asarray(r["d32_o"], dtype=np.float64)    # [128, 12]
        ccols = (np.arange(1, NBK) + 8 * k) % NB          # 38 global blocks
        np.add.at(S64, ccols, cs.T)
        acols = (np.arange(NA) + 8 * k) % NB              # 8 global blocks
        np.add.at(S64, acols, rs.reshape(128, NA, 3).sum(-1).T)
        a32 = 4 * k + np.arange(4)
        b32 = a32 + 32
        np.add.at(S64, a32, d32[:, 0:4].T)                # rs32
        np.add.at(S64, b32, d32[:, 4:8].T)                # cs32
        pos[a32] = d32[:, 8:12].T
        pos[b32] = d32[:, 8:12].T

    Sv = S64.reshape(TWOB)
    pv = pos.reshape(TWOB)
    tot = Sv - np.exp(2.0) + np.exp(2.0 * pv)
    loss = np.mean(np.log(tot) - 2.0 * pv)
    return np.float32(loss)


# revision 29
# speedup vs baseline: 1.2905x; 1.0600x over previous
"""NT-Xent loss kernel for Trainium2, 8-core SPMD — v6 (symmetric triangle).

Math (matches the reference):
  reps = concat(z_i, z_j)  [2B, C], B=4096, C=128; rhat = reps/|reps|
  S_r = sum_c exp(2 rhat_r . rhat_c);  pos_r = rhat_r . rhat_{(r+B)%2B}
  loss = mean_r( ln(S_r - e^2 + e^{2 pos_r}) - 2 pos_r )

exp(2 sim) is symmetric, so each unordered pair is exp'd ONCE:
  Rotation decomposition of the 64x64 block grid (blocks of 128 rows):
  every core runs the SAME canonical program computing tiles (a, a+d),
  a in 0..7, d in 0..31, on keys ROTATED by 8k blocks (host np.roll).
  Across 8 cores this covers every pair with |d| in 0..31 exactly once.
  The d=32 pairs (also the positive pairs) don't decompose under the
  rotation group; they are fed as separate per-core gathered inputs
  q32/p32 (4 tiles per core), whose diagonals give pos_r for free.

Per tile both sums are extracted: row sums ride ScalarE's accum_out;
column sums are accumulated elementwise into a bf16 strip A (DVE adds)
and partition-folded ONCE at the end on the PE (per-tile lhsT=A-tile
matmul against a ones vector). Host scatter-adds the per-core partials
(static maps), subtracts e^2 for the diagonal, and takes ln/mean in
fp64 over the 8192 rows.

Per core: 260 exp tiles (vs 512 full-matrix), 2.44MB keys DMA (only
blocks 0..38 are referenced canonically), bf16 matmuls (1024-wide
moving operand), normalization folded into the transposed copy via a
partition-broadcast row of 1/|k| (one DVE pass).
"""

import os

import numpy as np

import concourse.bacc as bacc
import concourse.bass as bass
import concourse.mybir as mybir
from concourse.bass_utils import run_bass_kernel_spmd
from concourse.masks import make_identity
from concourse.tile import TileContext

F32 = mybir.dt.float32
F32R = mybir.dt.float32r
BF16 = mybir.dt.bfloat16
AF = mybir.ActivationFunctionType
ALU = mybir.AluOpType
AX = mybir.AxisListType

B = 4096
C = 128
TWOB = 2 * B
N_CORES = 8
NB = 64                 # 128-row blocks in the full matrix
NBK = 39                # canonical key blocks each core loads (0..38)
KROWS = NBK * 128       # 4992
ISCALE = 2.0            # 1 / temperature
NA = 8                  # canonical stationary blocks per core (a = 0..7)
# keys DMA in 10 half-chunks, alternating rings so completion is ordered
DMAS = [(4 * h, 4 if 4 * h + 4 <= NBK else NBK - 4 * h) for h in range(10)]
GROUPS = [(0, 8), (8, 8), (16, 8), (24, 8), (32, 7)]  # norm/transpose groups


def _patch_act_tables():
    """Keep Exp/Ln only in natural_log_exp_and_others so bacc emits ONE
    ACT_TABLE_LOAD for the whole kernel."""
    if getattr(bacc, "_ntx_act_patched", False):
        return
    orig = bacc.get_activation_tables

    def patched(arch):
        out = {}
        for name, fns in orig(arch).items():
            if name != "natural_log_exp_and_others":
                fns = fns - {AF.Exp, AF.Ln}
            out[name] = fns
        return out

    bacc.get_activation_tables = patched
    bacc._ntx_act_patched = True


def build_bass() -> bass.Bass:
    _patch_act_tables()
    nc = bacc.Bacc()
    keys = nc.dram_tensor("keys", [KROWS, C], BF16, kind="ExternalInput")
    q32 = nc.dram_tensor("q32", [512, C], BF16, kind="ExternalInput")
    p32 = nc.dram_tensor("p32", [512, C], BF16, kind="ExternalInput")
    cs_o = nc.dram_tensor("cs_o", [128, NBK - 1], F32, kind="ExternalOutput")
    rs_o = nc.dram_tensor("rs_o", [128, 3 * NA], F32, kind="ExternalOutput")
    d32_o = nc.dram_tensor("d32_o", [128, 12], F32, kind="ExternalOutput")

    with TileContext(nc) as tc:
        with (
            tc.tile_pool(name="big", bufs=1) as big,
            tc.tile_pool(name="small", bufs=1) as small,
            tc.tile_pool(name="scr", bufs=2) as scr,
            tc.tile_pool(name="expp", bufs=4) as expp,
            tc.tile_pool(name="ps", bufs=2, space="PSUM") as psp,
            tc.tile_pool(name="psq", bufs=2, space="PSUM") as psq,
        ):
            # ---- identity/ones setup first (gpsimd), then key DMAs
            ident = small.tile([128, 128], F32)
            make_identity(nc, ident[:])
            identb = small.tile([128, 128], BF16)
            nc.vector.tensor_copy(identb[:], ident[:])
            onesb = small.tile([128, 1], BF16)
            nc.vector.memset(onesb[:], 1.0)

            # ---- input DMAs on sync+gpsimd rings (never ScalarE); half-
            # chunks alternate rings so chunks complete in order
            kt3 = big.tile([128, NBK, C], BF16)
            for h, (b0, nt) in enumerate(DMAS):
                eng = nc.sync if h % 2 == 0 else nc.gpsimd
                eng.dma_start(
                    out=kt3[:, b0 : b0 + nt, :],
                    in_=keys[b0 * 128 : (b0 + nt) * 128, :].rearrange(
                        "(t p) c -> p t c", p=128
                    ),
                )
            q32t = small.tile([128, 4, C], BF16)
            p32t = small.tile([128, 4, C], BF16)

            # cs accumulator strip: canonical key cols 1..38
            A = big.tile([128, (NBK - 1) * 128], BF16)
            nc.gpsimd.memset(A[:], 0.0)

            # ---- norms: nrm[:, 0:39] keys, 39:43 q32, 43:47 p32
            nrm = small.tile([128, 48], F32)
            inv = small.tile([128, 48], F32)

            def norms(x3, col, n):
                sq = scr.tile([128, 16, C], F32, tag="sq")
                nc.vector.tensor_mul(sq[:, 0:n, :], x3[:], x3[:])
                nc.vector.reduce_sum(nrm[:, col : col + n], sq[:, 0:n, :], axis=AX.X)

            def rsqrt_batch(col, n):
                nc.scalar.activation(nrm[:, col : col + n], nrm[:, col : col + n], AF.Ln)
                nc.scalar.activation(
                    inv[:, col : col + n], nrm[:, col : col + n], AF.Exp, scale=-0.5
                )

            keysT = big.tile([128, KROWS], BF16)
            kn3 = big.tile([128, NBK, C], BF16)

            def transpose_scale(g):
                b0, nt = GROUPS[g]
                nc.vector.tensor_mul(
                    kn3[:, b0 : b0 + nt, :],
                    kt3[:, b0 : b0 + nt, :],
                    inv[:, b0 : b0 + nt].unsqueeze(2).broadcast_to((128, nt, C)),
                )
                tq = psq.tile([128, 1024], BF16, tag="pq")
                for i in range(nt):
                    nc.tensor.transpose(
                        tq[:, i * 128 : (i + 1) * 128], kn3[:, b0 + i, :], identb[:]
                    )
                nc.vector.tensor_copy(
                    keysT[:, b0 * 128 : (b0 + nt) * 128], tq[:, 0 : nt * 128]
                )

            # head-critical: group 0 (blocks 0-7) first
            norms(kt3[:, 0:8, :], 0, 8)
            rsqrt_batch(0, 8)
            transpose_scale(0)

            rs = small.tile([128, 3 * NA], F32)

            # spans per a: p0 = d 0..7 (1024, incl diag), p1 = d 8..19
            # (1536), p2 = d 20..31 (1536).  p0 only needs key group 0.
            PARTS = [(0, 8), (8, 12), (20, 12)]

            def span(a, p):
                d0, nt = PARTS[p]
                c0 = (a + d0) * 128
                w = nt * 128
                psm = psp.tile([128, w], F32, tag="ps")
                lhs = keysT[:, a * 128 : (a + 1) * 128]
                for j in range(w // 512):
                    nc.tensor.matmul(
                        psm[:, j * 512 : (j + 1) * 512],
                        lhsT=lhs,
                        rhs=keysT[:, c0 + j * 512 : c0 + (j + 1) * 512],
                        start=True, stop=True,
                    )
                expb = expp.tile([128, w], BF16, tag="eb")
                nc.scalar.activation(
                    expb[:], psm[:], AF.Exp, scale=ISCALE,
                    accum_out=rs[:, 3 * a + p : 3 * a + p + 1],
                )
                # A cols (a+d0)..(a+d0+nt-1), diag tile excluded for p0
                if p == 0:
                    nc.vector.tensor_add(
                        A[:, a * 128 : (a + 7) * 128],
                        A[:, a * 128 : (a + 7) * 128],
                        expb[:, 128:1024],
                    )
                else:
                    lo = (a + d0 - 1) * 128
                    nc.vector.tensor_add(
                        A[:, lo : lo + w], A[:, lo : lo + w], expb[:]
                    )

            d32out = small.tile([128, 12], F32)
            exp32 = small.tile([128, 512], BF16)

            def d32_dma():
                nc.gpsimd.dma_start(
                    out=q32t[:], in_=q32[:].rearrange("(t p) c -> p t c", p=128)
                )
                nc.sync.dma_start(
                    out=p32t[:], in_=p32[:].rearrange("(t p) c -> p t c", p=128)
                )

            def d32_prep():
                norms(q32t, 39, 4)
                norms(p32t, 43, 4)
                rsqrt_batch(39, 8)
                q32n = scr.tile([128, 4, C], BF16, tag="d32n")
                p32n = scr.tile([128, 4, C], BF16, tag="d32n")
                nc.vector.tensor_mul(
                    q32n[:], q32t[:],
                    inv[:, 39:43].unsqueeze(2).broadcast_to((128, 4, C)),
                )
                nc.vector.tensor_mul(
                    p32n[:], p32t[:],
                    inv[:, 43:47].unsqueeze(2).broadcast_to((128, 4, C)),
                )
                tp = psq.tile([128, 1024], BF16, tag="pq")
                for t in range(4):
                    nc.tensor.transpose(
                        tp[:, t * 128 : (t + 1) * 128], q32n[:, t, :], identb[:]
                    )
                for t in range(4):
                    nc.tensor.transpose(
                        tp[:, 512 + t * 128 : 512 + (t + 1) * 128], p32n[:, t, :],
                        identb[:],
                    )
                qpT = small.tile([128, 1024], BF16)
                nc.vector.tensor_copy(qpT[:], tp[:])
                return qpT

            def d32_main(qpT):
                psm32 = psq.tile([128, 512], F32, tag="pq")
                for t in range(4):
                    nc.tensor.matmul(
                        psm32[:, t * 128 : (t + 1) * 128],
                        lhsT=qpT[:, t * 128 : (t + 1) * 128],
                        rhs=qpT[:, 512 + t * 128 : 512 + (t + 1) * 128],
                        start=True, stop=True,
                    )
                # pos32 = diagonals of the 4 normalized sim tiles
                dsc = scr.tile([128, 4, 128], F32, tag="dsc")
                nc.vector.tensor_mul(
                    dsc[:],
                    psm32[:].rearrange("p (t f) -> p t f", t=4),
                    ident[:].unsqueeze(1).broadcast_to((128, 4, 128)),
                )
                nc.vector.reduce_sum(d32out[:, 8:12], dsc[:], axis=AX.X)
                nc.scalar.activation(exp32[:], psm32[:], AF.Exp, scale=ISCALE)
                # rs32: free-axis sums on DVE
                nc.vector.reduce_sum(
                    d32out[:, 0:4],
                    exp32[:].rearrange("p (t f) -> p t f", t=4),
                    axis=AX.X,
                )
                # cs32: partition sums via PE fold
                csp32 = psq.tile([128, 4], F32, tag="pq")
                for t in range(4):
                    nc.tensor.matmul(
                        csp32[:, t : t + 1],
                        lhsT=exp32[:, t * 128 : (t + 1) * 128],
                        rhs=onesb[:],
                        start=True, stop=True,
                    )
                nc.vector.tensor_copy(d32out[:, 4:8], csp32[:])

            # ---- progressive A folds (A-idx j final after its last
            # contributing span): p0 touches j in [a, a+6], p1 [a+7, a+18],
            # p2 [a+19, a+30].
            cs_sb = small.tile([128, NBK - 1], F32)

            def fold(c_lo, c_hi):
                csp = psq.tile([128, c_hi - c_lo], F32, tag="pq")
                for c in range(c_lo, c_hi):
                    nc.tensor.matmul(
                        csp[:, c - c_lo : c - c_lo + 1],
                        lhsT=A[:, c * 128 : (c + 1) * 128],
                        rhs=onesb[:],
                        start=True, stop=True,
                    )
                nc.vector.tensor_copy(cs_sb[:, c_lo:c_hi], csp[:])

            # ---- main: per-group prep feeds wave1 (p0, group 0-1
            # only), then wave2 (p1), wave3 (p2); folds as cols finalize
            norms(kt3[:, 8:16, :], 8, 8)
            rsqrt_batch(8, 8)
            transpose_scale(1)
            span(0, 0)
            span(1, 0)
            norms(kt3[:, 16:24, :], 16, 8)
            span(2, 0)
            rsqrt_batch(16, 8)
            span(3, 0)
            transpose_scale(2)
            d32_dma()
            span(4, 0)
            norms(kt3[:, 24:32, :], 24, 8)
            span(5, 0)
            rsqrt_batch(24, 8)
            span(6, 0)
            transpose_scale(3)
            span(7, 0)
            fold(0, 7)
            norms(kt3[:, 32:39, :], 32, 7)
            rsqrt_batch(32, 7)
            transpose_scale(4)
            span(0, 1)
            span(1, 1)
            qpT = d32_prep()
            span(2, 1)
            span(3, 1)
            span(4, 1)
            d32_main(qpT)
            nc.gpsimd.dma_start(out=d32_o[:], in_=d32out[:])
            span(5, 1)
            span(6, 1)
            span(7, 1)
            fold(7, 19)
            span(0, 2)
            span(1, 2)
            span(2, 2)
            span(3, 2)
            span(4, 2)
            span(5, 2)
            span(6, 2)
            fold(19, 26)
            span(7, 2)
            nc.gpsimd.dma_start(out=rs_o[:], in_=rs[:])
            fold(26, NBK - 1)
            nc.sync.dma_start(out=cs_o[:], in_=cs_sb[:])

    nc.finalize()
    return nc


_NC_CACHE: bass.Bass | None = None
LAST_RESULTS = None  # BassKernelResults of the last run (for profiling)


def _get_nc() -> bass.Bass:
    global _NC_CACHE
    if _NC_CACHE is None:
        _NC_CACHE = build_bass()
    return _NC_CACHE


def kernel(z_i: np.ndarray, z_j: np.ndarray) -> np.ndarray:
    global LAST_RESULTS
    z_i = np.asarray(z_i, dtype=np.float32)
    z_j = np.asarray(z_j, dtype=np.float32)
    assert z_i.shape == (B, C) and z_j.shape == (B, C)

    import ml_dtypes
    reps = np.concatenate([z_i, z_j], axis=0)  # [2B, C]
    reps64 = reps.reshape(NB, 128, C).astype(ml_dtypes.bfloat16)
    in_maps = []
    for k in range(N_CORES):
        rot = np.roll(reps64, -8 * k, axis=0)
        in_maps.append(
            {
                "keys": np.ascontiguousarray(rot[:NBK].reshape(KROWS, C)),
                "q32": np.ascontiguousarray(
                    reps64[4 * k : 4 * k + 4].reshape(512, C)
                ),
                "p32": np.ascontiguousarray(
                    reps64[4 * k + 32 : 4 * k + 36].reshape(512, C)
                ),
            }
        )

    nc = _get_nc()
    trace = bool(int(os.environ.get("KERNEL_TRACE", "0")))
    res = run_bass_kernel_spmd(
        nc, in_maps, core_ids=list(range(N_CORES)), trace=trace
    )
    LAST_RESULTS = res

    # ---- host assembly (tiny: 8192-row scatter + ln in fp64)
    S64 = np.zeros((NB, 128), dtype=np.float64)
    pos = np.zeros((NB, 128), dtype=np.float64)
    for k in range(N_CORES):
        r = res.results[k]
        cs = np.asarray(r["cs_o"], dtype=np.float64)      # [128, 38]
        rs = np.asarray(r["rs_o"], dtype=np.float64)      # [128, 24]
        d32 = np.asarray(r["d32_o"], dtype=np.float64)    # [128, 12]
        ccols = (np.arange(1, NBK) + 8 * k) % NB          # 38 global blocks
        np.add.at(S64, ccols, cs.T)
        acols = (np.arange(NA) + 8 * k) % NB              # 8 global blocks
        np.add.at(S64, acols, rs.reshape(128, NA, 3).sum(-1).T)
        a32 = 4 * k + np.arange(4)
        b32 = a32 + 32
        np.add.at(S64, a32, d32[:, 0:4].T)                # rs32
        np.add.at(S64, b32, d32[:, 4:8].T)                # cs32
        pos[a32] = d32[:, 8:12].T
        pos[b32] = d32[:, 8:12].T

    Sv = S64.reshape(TWOB)
    pv = pos.reshape(TWOB)
    tot = Sv - np.exp(2.0) + np.exp(2.0 * pv)
    loss = np.mean(np.log(tot) - 2.0 * pv)
    return np.float32(loss)
